# revision 10
# baseline (speedup 1.0000x reference)
"""Bass/Trainium2 kernel for nn_BiGRU_1486058685008.

Reference computation: patchify(x, 16) -> 3-layer bidirectional GRU
(PyTorch gate order r,z,n), input (16, 3, 512, 512) -> (16, 1024, 768).

Strategy: data-parallel over batch across the 8 NeuronCores (B_local=2 per
core); each core runs both directions and all 3 layers locally so no
cross-core communication is needed.  Within a core:

 - Input-gate projections xg = x @ W_ih^T + biases are big fp32 matmuls
   (W_ih^T stationary tiles, tokens moving), evacuated with a per-partition
   bias add and staged to DRAM in [S, P, 9*BL] layout.
 - The recurrence keeps gates TRANSPOSED: gates^T (1152 gate dims over 9
   partition-chunks, B_local free).  Stationary = W_hh^T tiles in bf16 (FWL
   double-rate weight loads), moving = h^T bf16 (tiny N).  PSUM holds
   (128, 9*BL) per step; elementwise runs on short (128, *) tiles.
   h is kept fp32; a bf16 shadow feeds the next matmul.
 - The For_i body covers 2*UNROLL steps (two halves).  All per-step
   addressing is static: xg is fetched one half (UNROLL steps) at a time
   with a single dynamic-offset DMA into a double-buffered SBUF block, and
   h is written into an SBUF strip that is flushed to DRAM once per half.
   This keeps dynamic-AP register pressure tiny (the per-engine register
   files cannot hold per-step dynamic offsets for an unrolled body).
 - Backward-direction data uses natural time order everywhere; the bwd
   half simply addresses its strip/ring slots reversed (statically).
"""

import contextlib

import numpy as np
import ml_dtypes

import concourse.bass as bass
import concourse.mybir as mybir
import concourse.tile as tile
from concourse import bacc
from concourse.bass import ds, ts

F32 = mybir.dt.float32
BF16 = mybir.dt.bfloat16
AF = mybir.ActivationFunctionType

P = 128
D = 768         # input dim of every layer (patch dim; 2H for layers 1,2)
H = 384         # hidden per direction
G3 = 3 * H      # 1152
KC_N = D // P   # 6 contraction chunks for projections
MC_N = G3 // P  # 9 gate chunks
HC_N = H // P   # 3 hidden chunks
DEPTH = 3
S_FULL = 1024
B_FULL = 16
N_CORES = 8



# ---------------------------------------------------------------------------
# Kernel builder
# ---------------------------------------------------------------------------

def build_kernel(S=S_FULL, BL=B_FULL // N_CORES, U=16):
    """Build the per-core Bass program.

    S: sequence length, BL: local batch, U: steps per half-body (the For_i
    body covers 2*U steps).  xg DRAM rows are shifted by U: time s lives at
    row s+U, so both directions' half-block prefetch stays in bounds.
    """
    assert S % (2 * U) == 0
    T = S * BL
    CB = MC_N * BL          # columns per xg step block (9*BL)
    NW = 3 * BL             # one gate region width
    RZ = 6 * BL             # r+z region width
    TOK = 512 if T % 512 == 0 else T // 2
    assert T % TOK == 0 and TOK % BL == 0
    TB_N = T // TOK
    S_TOK = TOK // BL       # s-steps covered by one token block
    NIT = S // (2 * U)

    nc = bacc.Bacc()

    patchesT = nc.declare_dram_parameter("patchesT", [D, T], F32, isOutput=False)
    wihT = nc.declare_dram_parameter("wihT", [DEPTH, 2, D, G3], F32, isOutput=False)
    whhT = nc.declare_dram_parameter("whhT", [DEPTH, 2, H, G3], BF16, isOutput=False)
    # bias9[l,d,p,m]: m<6 -> (b_ih+b_hh) for r,z chunks; m>=6 -> b_ih n chunks
    bias9 = nc.declare_dram_parameter("bias9", [DEPTH, 2, P, MC_N], F32, isOutput=False)
    # bhnb[l,d,p,c*BL+b] = b_hh n-gate, broadcast over local batch
    bhnb = nc.declare_dram_parameter("bhnb", [DEPTH, 2, P, NW], F32, isOutput=False)

    # outputs in device layout [s, p, c*BL+b]; host reassembles
    out_f = nc.declare_dram_parameter("out_f", [S, P, NW], F32, isOutput=True)
    out_b = nc.declare_dram_parameter("out_b", [S, P, NW], F32, isOutput=True)

    # Dynamic-offset DMAs consume per-engine registers that are never
    # recycled (~4-5 each, 54 per engine); round-robin them across engine
    # queues so no single engine's register file is exhausted.
    _dyn = {"i": 0}

    def dyn_dma(nc, out, in_):
        engs = [nc.sync, nc.scalar, nc.gpsimd]
        e = engs[_dyn["i"] % len(engs)]
        _dyn["i"] += 1
        e.dma_start(out, in_)

    with tile.TileContext(nc) as tc, contextlib.ExitStack() as ctx:
        consts = ctx.enter_context(tc.tile_pool(name="consts", bufs=1))
        wih_pool = ctx.enter_context(tc.tile_pool(name="wih", bufs=6))
        rhs_pool = ctx.enter_context(tc.tile_pool(name="rhs", bufs=1))
        stage_pool = ctx.enter_context(tc.tile_pool(name="stage", bufs=3))
        psum_proj = ctx.enter_context(
            tc.tile_pool(name="psum_proj", bufs=4, space="PSUM"))
        psum_rec = ctx.enter_context(
            tc.tile_pool(name="psum_rec", bufs=1, space="PSUM"))
        dram = ctx.enter_context(tc.tile_pool(name="dram", bufs=1, space="DRAM"))
        work = ctx.enter_context(tc.tile_pool(name="work", bufs=1))

        # persistent DRAM scratch
        xg_dram = [[dram.tile([S + 2 * U, P, CB], F32,
                              tag=f"xg{l}d{d}", name=f"xg{l}d{d}")
                    for d in range(2)] for l in range(DEPTH)]
        h_hist = [[dram.tile([S, P, NW], F32, tag=f"hh{l}d{d}",
                             name=f"hh{l}d{d}")
                   for d in range(2)] for l in range(2)]

        # zero-fill the xg slack rows (prefetch overruns read them; the
        # values are never consumed but must not be NaN for the simulator)
        zf = work.tile([P, U, CB], F32, tag="zf", name="zf")
        nc.vector.memset(zf[:], 0.0)
        for l in range(DEPTH):
            for d in range(2):
                for r0 in (0, S + U):
                    nc.sync.dma_start(
                        xg_dram[l][d][r0:r0 + U].rearrange("s p c -> p s c"),
                        zf[:])

        for l in range(DEPTH):
            last = (l == DEPTH - 1)

            # ---------------- per-layer constants ----------------
            whh_sb, bias9_sb, bhnb_sb = [], [], []
            for d in range(2):
                w = consts.tile([P, HC_N, G3], BF16, tag=f"whh{d}",
                                name=f"whh{d}")
                nc.sync.dma_start(
                    w[:], whhT[l, d].rearrange("(k p) g -> p k g", p=P))
                whh_sb.append(w)
                b9 = consts.tile([P, MC_N], F32, tag=f"b9{d}", name=f"b9{d}")
                nc.sync.dma_start(b9[:], bias9[l, d])
                bias9_sb.append(b9)
                bh = consts.tile([P, NW], F32, tag=f"bh{d}", name=f"bh{d}")
                nc.sync.dma_start(bh[:], bhnb[l, d])
                bhnb_sb.append(bh)

            # ---------------- projection phase ----------------
            # moving operand tiles (128, TOK), all resident for the layer
            rhs_tiles = {}
            for kc in range(KC_N):
                for tb in range(TB_N):
                    rt = rhs_pool.tile([P, TOK], F32, tag=f"rhs{kc}_{tb}",
                                       name=f"rhs{kc}_{tb}")
                    if l == 0:
                        nc.sync.dma_start(
                            rt[:], patchesT[ts(kc, P), ts(tb, TOK)])
                    else:
                        src = h_hist[l - 1][0] if kc < HC_N else h_hist[l - 1][1]
                        c = kc % HC_N
                        s0 = tb * S_TOK
                        nc.sync.dma_start(
                            rt[:].rearrange("p (s b) -> p s b", b=BL),
                            src[s0:s0 + S_TOK, :, ts(c, BL)]
                            .rearrange("s p b -> p s b"))
                    rhs_tiles[(kc, tb)] = rt

            for d in range(2):
                for m in range(MC_N):
                    wts = []
                    for kc in range(KC_N):
                        wt = wih_pool.tile([P, P], F32, tag="wt",
                                           name="wt")
                        nc.sync.dma_start(
                            wt[:], wihT[l, d, ts(kc, P), ts(m, P)])
                        wts.append(wt)
                    pss = [psum_proj.tile([P, TOK], F32, name="psp")
                           for _ in range(TB_N)]
                    for kc in range(KC_N):
                        for tb in range(TB_N):
                            nc.tensor.matmul(
                                pss[tb][:], wts[kc][:], rhs_tiles[(kc, tb)][:],
                                start=(kc == 0), stop=(kc == KC_N - 1))
                    for tb in range(TB_N):
                        st = stage_pool.tile([P, TOK], F32, tag="evac",
                                             name="evac")
                        nc.vector.tensor_scalar_add(
                            st[:], pss[tb][:], bias9_sb[d][:, m:m + 1])
                        s0 = tb * S_TOK + U
                        nc.sync.dma_start(
                            xg_dram[l][d][s0:s0 + S_TOK, :,
                                          m * BL:(m + 1) * BL]
                            .rearrange("s p b -> p s b"),
                            st[:].rearrange("p (s b) -> p s b", b=BL))

            # ---------------- recurrence phase ----------------
            # xg ring: one half (U steps) per buffer, double-buffered
            xg_sb = [[work.tile([P, U, CB], F32, tag=f"xg{d}h{hp}",
                                name=f"xg{d}h{hp}")
                      for hp in range(2)] for d in range(2)]
            # h strips: fp32 + bf16 shadow, written per-step, flushed per half
            hst = [[work.tile([P, U, NW], F32, tag=f"hs{d}h{hp}",
                              name=f"hs{d}h{hp}")
                    for hp in range(2)] for d in range(2)]
            hbf = [[work.tile([P, U, NW], BF16, tag=f"hb{d}h{hp}",
                              name=f"hb{d}h{hp}")
                    for hp in range(2)] for d in range(2)]
            trz = [work.tile([P, RZ], F32, tag=f"trz{d}", name=f"trz{d}")
                   for d in range(2)]
            tn = [work.tile([P, NW], F32, tag=f"tn{d}", name=f"tn{d}")
                  for d in range(2)]
            tn2 = [work.tile([P, NW], F32, tag=f"tn2{d}", name=f"tn2{d}")
                   for d in range(2)]
            psr = [[psum_rec.tile([P, CB], F32, tag=f"ps{d}p{par}",
                                  name=f"ps{d}p{par}")
                    for par in range(2)] for d in range(2)]

            for d in range(2):
                # initial h (=0) lives in strip[1] at the slot the first
                # step reads: fwd reads [1][U-1], bwd reads [1][0]
                nc.vector.memset(hst[d][1][:], 0.0)
                nc.vector.memset(hbf[d][1][:], 0.0)
                # prologue xg prefetch for half 0 of iteration 0
                row0 = U if d == 0 else S
                nc.sync.dma_start(
                    xg_sb[d][0][:],
                    xg_dram[l][d][row0:row0 + U].rearrange("s p c -> p s c"))

            with tc.For_i(0, NIT, 1,
                          hint_engines=(mybir.EngineType.PE,
                                        mybir.EngineType.DVE,
                                        mybir.EngineType.Activation,
                                        mybir.EngineType.SP)) as it:
                g00 = it * (2 * U)
                for half in range(2):
                    hp = half
                    for d in range(2):
                        # prefetch next half's xg into the other buffer
                        if d == 0:
                            row_n = g00 + (2 + half) * U
                        else:
                            row_n = (S - (1 + half) * U) - g00
                        dyn_dma(nc,
                            xg_sb[d][1 - hp][:],
                            xg_dram[l][d][ds(row_n, U), :, :]
                            .rearrange("s p c -> p s c"))

                    for u in range(U):
                        par = u % 2
                        for d in range(2):
                            # strip slots (static):
                            w_slot = u if d == 0 else U - 1 - u
                            if u == 0:
                                rt_t, r_slot = hst[d][1 - hp], (U - 1 if d == 0 else 0)
                                rb_t = hbf[d][1 - hp]
                            else:
                                rt_t = hst[d][hp]
                                rb_t = hbf[d][hp]
                                r_slot = u - 1 if d == 0 else U - u
                            xslot = u if d == 0 else U - 1 - u
                            ps = psr[d][par]
                            xg = xg_sb[d][hp][:, xslot, :]
                            hprev = rt_t[:, r_slot, :]
                            hbprev = rb_t[:, r_slot, :]
                            hnew = hst[d][hp][:, w_slot, :]
                            hbnew = hbf[d][hp][:, w_slot, :]

                            for m in range(MC_N):
                                for k in range(HC_N):
                                    nc.tensor.matmul(
                                        ps[:, m * BL:(m + 1) * BL],
                                        whh_sb[d][:, k, ts(m, P)],
                                        hbprev[:, ts(k, BL)],
                                        start=(k == 0), stop=(k == HC_N - 1))

                            nc.vector.tensor_add(
                                trz[d][:], ps[:, :RZ], xg[:, :RZ])
                            nc.scalar.activation(
                                trz[d][:], trz[d][:], AF.Sigmoid)
                            nc.vector.tensor_add(
                                tn[d][:], ps[:, RZ:CB], bhnb_sb[d][:])
                            nc.vector.tensor_mul(
                                tn[d][:], trz[d][:, :NW], tn[d][:])
                            nc.vector.tensor_add(
                                tn[d][:], tn[d][:], xg[:, RZ:CB])
                            nc.scalar.activation(tn[d][:], tn[d][:], AF.Tanh)
                            nc.vector.tensor_sub(
                                tn2[d][:], hprev[:], tn[d][:])
                            nc.vector.tensor_mul(
                                tn2[d][:], trz[d][:, NW:RZ], tn2[d][:])
                            nc.vector.tensor_add(hnew[:], tn[d][:], tn2[d][:])
                            nc.vector.tensor_copy(hbnew[:], hnew[:])

                    # flush h strips to DRAM (one DMA per dir per half)
                    for d in range(2):
                        dst = (out_f if d == 0 else out_b) if last \
                            else h_hist[l][d]
                        if d == 0:
                            roww = g00 + half * U
                        else:
                            roww = (S - (1 + half) * U) - g00
                        dyn_dma(nc,
                            dst[ds(roww, U), :, :].rearrange("s p c -> p s c"),
                            hst[d][hp][:])
    nc.compile()
    return nc


# ---------------------------------------------------------------------------
# Host side
# ---------------------------------------------------------------------------

def _patchify(x, p=16):
    b, c, W, Hh = x.shape
    w, h = W // p, Hh // p
    x = x.reshape(b, c, w, p, h, p)
    x = x.transpose(0, 2, 4, 3, 5, 1)
    return x.reshape(b, w * h, p * p * c)


def pack_weights(W_ih, W_hh, b_ih, b_hh, BL):
    wihT = np.ascontiguousarray(
        np.asarray(W_ih, np.float32).transpose(0, 1, 3, 2))
    whhT = np.ascontiguousarray(
        np.asarray(W_hh, np.float32).transpose(0, 1, 3, 2)).astype(
            ml_dtypes.bfloat16)
    b_ih = np.asarray(b_ih, np.float32)
    b_hh = np.asarray(b_hh, np.float32)
    brz = b_ih[:, :, :2 * H] + b_hh[:, :, :2 * H]
    bn_i = b_ih[:, :, 2 * H:]
    bias9 = np.concatenate(
        [brz.reshape(DEPTH, 2, 6, P), bn_i.reshape(DEPTH, 2, 3, P)], axis=2)
    bias9 = np.ascontiguousarray(bias9.transpose(0, 1, 3, 2))  # [l,d,p,m]
    bhn = b_hh[:, :, 2 * H:].reshape(DEPTH, 2, 3, 1, P)
    bhnb = np.broadcast_to(bhn, (DEPTH, 2, 3, BL, P)).reshape(
        DEPTH, 2, 3 * BL, P)
    bhnb = np.ascontiguousarray(bhnb.transpose(0, 1, 3, 2))    # [l,d,p,c*BL+b]
    return wihT, whhT, bias9, bhnb


def host_inputs(x, W_ih, W_hh, b_ih, b_hh, n_cores=N_CORES):
    patches = _patchify(np.asarray(x, np.float32))      # (B, S, D)
    B = patches.shape[0]
    BL = B // n_cores
    wihT, whhT, bias9, bhnb = pack_weights(W_ih, W_hh, b_ih, b_hh, BL)
    in_maps = []
    for cc in range(n_cores):
        pt = patches[cc * BL:(cc + 1) * BL]             # (BL, S, D)
        ptT = np.ascontiguousarray(pt.transpose(2, 1, 0)).reshape(D, -1)
        in_maps.append({
            "patchesT": ptT,
            "wihT": wihT,
            "whhT": whhT,
            "bias9": bias9,
            "bhnb": bhnb,
        })
    return in_maps


def unpack_out(res, S, BL):
    """Device out layout [s, p, c*BL+b] -> (BL, S, H)."""
    o = np.asarray(res).reshape(S, P, HC_N, BL)
    return np.ascontiguousarray(o.transpose(3, 0, 2, 1)).reshape(BL, S, H)


_CACHE = {}


def get_runner():
    """Build + compile once; returns a callable over in_maps."""
    if "runner" not in _CACHE:
        from concourse.bass_utils import run_bass_kernel_spmd
        nc = build_kernel()
        _CACHE["nc"] = nc

        def runner(in_maps):
            return run_bass_kernel_spmd(
                nc, in_maps, core_ids=list(range(N_CORES))).results
        _CACHE["runner"] = runner
    return _CACHE["runner"]


def kernel(x, W_ih, W_hh, b_ih, b_hh):
    in_maps = host_inputs(x, W_ih, W_hh, b_ih, b_hh)
    results = get_runner()(in_maps)
    BL = B_FULL // N_CORES
    y = np.empty((B_FULL, S_FULL, 2 * H), np.float32)
    for cc in range(N_CORES):
        y[cc * BL:(cc + 1) * BL, :, :H] = unpack_out(
            results[cc]["out_f"], S_FULL, BL)
        y[cc * BL:(cc + 1) * BL, :, H:] = unpack_out(
            results[cc]["out_b"], S_FULL, BL)
    return y


# revision 12
# speedup vs baseline: 18.7035x; 18.7035x over previous
"""Bass/Trainium2 kernel for nn_BiGRU_1486058685008.

Reference computation: patchify(x, 16) -> 3-layer bidirectional GRU
(PyTorch gate order r,z,n), input (16, 3, 512, 512) -> (16, 1024, 768).

Strategy: data-parallel over batch across the 8 NeuronCores (B_local=2 per
core); each core runs both directions and all 3 layers locally so no
cross-core communication is needed.  Within a core:

 - Input-gate projections xg = x @ W_ih^T + biases are big fp32 matmuls
   (W_ih^T stationary tiles, tokens moving), evacuated with a per-partition
   bias add and staged to DRAM in [S, P, 9*BL] layout.
 - The recurrence keeps gates TRANSPOSED: gates^T (1152 gate dims over 9
   partition-chunks, B_local free).  Stationary = W_hh^T tiles in bf16 (FWL
   double-rate weight loads), moving = h^T bf16 (tiny N).  PSUM holds
   (128, 9*BL) per step; elementwise runs on short (128, *) tiles.
   h is kept fp32; a bf16 shadow feeds the next matmul.
 - The For_i body covers 2*UNROLL steps (two halves).  All per-step
   addressing is static: xg is fetched one half (UNROLL steps) at a time
   with a single dynamic-offset DMA into a double-buffered SBUF block, and
   h is written into an SBUF strip that is flushed to DRAM once per half.
   This keeps dynamic-AP register pressure tiny (the per-engine register
   files cannot hold per-step dynamic offsets for an unrolled body).
 - Backward-direction data uses natural time order everywhere; the bwd
   half simply addresses its strip/ring slots reversed (statically).
"""

import contextlib

import numpy as np
import ml_dtypes

import concourse.bass as bass
import concourse.mybir as mybir
import concourse.tile as tile
from concourse import bacc
from concourse.bass import ds, ts

F32 = mybir.dt.float32
BF16 = mybir.dt.bfloat16
AF = mybir.ActivationFunctionType

P = 128
D = 768         # input dim of every layer (patch dim; 2H for layers 1,2)
H = 384         # hidden per direction
G3 = 3 * H      # 1152
KC_N = D // P   # 6 contraction chunks for projections
MC_N = G3 // P  # 9 gate chunks
HC_N = H // P   # 3 hidden chunks
DEPTH = 3
S_FULL = 1024
B_FULL = 16
N_CORES = 8



# ---------------------------------------------------------------------------
# Kernel builder
# ---------------------------------------------------------------------------

def build_kernel(S=S_FULL, BL=B_FULL // N_CORES, U=16):
    """Build the per-core Bass program.

    S: sequence length, BL: local batch, U: steps per half-body (the For_i
    body covers 2*U steps).  xg DRAM rows are shifted by U: time s lives at
    row s+U, so both directions' half-block prefetch stays in bounds.
    """
    assert S % (2 * U) == 0
    T = S * BL
    CB = MC_N * BL          # columns per xg step block (9*BL)
    NW = 3 * BL             # one gate region width
    RZ = 6 * BL             # r+z region width
    TOK = 512 if T % 512 == 0 else T // 2
    assert T % TOK == 0 and TOK % BL == 0
    TB_N = T // TOK
    S_TOK = TOK // BL       # s-steps covered by one token block
    NIT = S // (2 * U)

    nc = bacc.Bacc()

    patchesT = nc.declare_dram_parameter("patchesT", [D, T], F32, isOutput=False)
    wihT = nc.declare_dram_parameter("wihT", [DEPTH, 2, D, G3], F32, isOutput=False)
    whhT = nc.declare_dram_parameter("whhT", [DEPTH, 2, H, G3], BF16, isOutput=False)
    # bias9[l,d,p,m]: m<6 -> (b_ih+b_hh) for r,z chunks; m>=6 -> b_ih n chunks
    bias9 = nc.declare_dram_parameter("bias9", [DEPTH, 2, P, MC_N], F32, isOutput=False)
    # bhnb[l,d,p,c*BL+b] = b_hh n-gate, broadcast over local batch
    bhnb = nc.declare_dram_parameter("bhnb", [DEPTH, 2, P, NW], F32, isOutput=False)

    # outputs in device layout [s, p, c*BL+b]; host reassembles
    out_f = nc.declare_dram_parameter("out_f", [S, P, NW], F32, isOutput=True)
    out_b = nc.declare_dram_parameter("out_b", [S, P, NW], F32, isOutput=True)

    # Dynamic-offset DMAs consume per-engine registers that are never
    # recycled (~4-5 each, 54 per engine); round-robin them across engine
    # queues so no single engine's register file is exhausted.
    _dyn = {"i": 0}

    def dyn_dma(nc, out, in_):
        engs = [nc.sync, nc.scalar, nc.gpsimd]
        e = engs[_dyn["i"] % len(engs)]
        _dyn["i"] += 1
        e.dma_start(out, in_)

    with tile.TileContext(nc) as tc, contextlib.ExitStack() as ctx:
        consts = ctx.enter_context(tc.tile_pool(name="consts", bufs=1))
        wih_pool = ctx.enter_context(tc.tile_pool(name="wih", bufs=6))
        rhs_pool = ctx.enter_context(tc.tile_pool(name="rhs", bufs=1))
        stage_pool = ctx.enter_context(tc.tile_pool(name="stage", bufs=3))
        psum_proj = ctx.enter_context(
            tc.tile_pool(name="psum_proj", bufs=4, space="PSUM"))
        psum_rec = ctx.enter_context(
            tc.tile_pool(name="psum_rec", bufs=1, space="PSUM"))
        dram = ctx.enter_context(tc.tile_pool(name="dram", bufs=1, space="DRAM"))
        work = ctx.enter_context(tc.tile_pool(name="work", bufs=1))

        # persistent DRAM scratch
        xg_dram = [[dram.tile([S + 2 * U, P, CB], F32,
                              tag=f"xg{l}d{d}", name=f"xg{l}d{d}")
                    for d in range(2)] for l in range(DEPTH)]
        h_hist = [[dram.tile([S, P, NW], F32, tag=f"hh{l}d{d}",
                             name=f"hh{l}d{d}")
                   for d in range(2)] for l in range(2)]

        # zero-fill the xg slack rows (prefetch overruns read them; the
        # values are never consumed but must not be NaN for the simulator)
        zf = work.tile([P, U, CB], F32, tag="zf", name="zf")
        nc.vector.memset(zf[:], 0.0)
        for l in range(DEPTH):
            for d in range(2):
                for r0 in (0, S + U):
                    nc.sync.dma_start(
                        xg_dram[l][d][r0:r0 + U].rearrange("s p c -> p s c"),
                        zf[:])

        for l in range(DEPTH):
            last = (l == DEPTH - 1)

            # ---------------- per-layer constants ----------------
            whh_sb, bias9_sb, bhnb_sb = [], [], []
            for d in range(2):
                w = consts.tile([P, HC_N, G3], BF16, tag=f"whh{d}",
                                name=f"whh{d}")
                nc.sync.dma_start(
                    w[:], whhT[l, d].rearrange("(k p) g -> p k g", p=P))
                whh_sb.append(w)
                b9 = consts.tile([P, MC_N], F32, tag=f"b9{d}", name=f"b9{d}")
                nc.sync.dma_start(b9[:], bias9[l, d])
                bias9_sb.append(b9)
                bh = consts.tile([P, NW], F32, tag=f"bh{d}", name=f"bh{d}")
                nc.sync.dma_start(bh[:], bhnb[l, d])
                bhnb_sb.append(bh)

            # ---------------- projection phase ----------------
            # moving operand tiles (128, TOK), all resident for the layer
            rhs_tiles = {}
            for kc in range(KC_N):
                for tb in range(TB_N):
                    rt = rhs_pool.tile([P, TOK], F32, tag=f"rhs{kc}_{tb}",
                                       name=f"rhs{kc}_{tb}")
                    if l == 0:
                        nc.sync.dma_start(
                            rt[:], patchesT[ts(kc, P), ts(tb, TOK)])
                    else:
                        src = h_hist[l - 1][0] if kc < HC_N else h_hist[l - 1][1]
                        c = kc % HC_N
                        s0 = tb * S_TOK
                        nc.sync.dma_start(
                            rt[:].rearrange("p (s b) -> p s b", b=BL),
                            src[s0:s0 + S_TOK, :, ts(c, BL)]
                            .rearrange("s p b -> p s b"))
                    rhs_tiles[(kc, tb)] = rt

            for d in range(2):
                for m in range(MC_N):
                    wts = []
                    for kc in range(KC_N):
                        wt = wih_pool.tile([P, P], F32, tag="wt",
                                           name="wt")
                        nc.sync.dma_start(
                            wt[:], wihT[l, d, ts(kc, P), ts(m, P)])
                        wts.append(wt)
                    pss = [psum_proj.tile([P, TOK], F32, name="psp")
                           for _ in range(TB_N)]
                    for kc in range(KC_N):
                        for tb in range(TB_N):
                            nc.tensor.matmul(
                                pss[tb][:], wts[kc][:], rhs_tiles[(kc, tb)][:],
                                start=(kc == 0), stop=(kc == KC_N - 1))
                    for tb in range(TB_N):
                        st = stage_pool.tile([P, TOK], F32, tag="evac",
                                             name="evac")
                        nc.vector.tensor_scalar_add(
                            st[:], pss[tb][:], bias9_sb[d][:, m:m + 1])
                        s0 = tb * S_TOK + U
                        nc.sync.dma_start(
                            xg_dram[l][d][s0:s0 + S_TOK, :,
                                          m * BL:(m + 1) * BL]
                            .rearrange("s p b -> p s b"),
                            st[:].rearrange("p (s b) -> p s b", b=BL))

            # ---------------- recurrence phase ----------------
            # xg ring: one half (U steps) per buffer, double-buffered
            xg_sb = [[work.tile([P, U, CB], F32, tag=f"xg{d}h{hp}",
                                name=f"xg{d}h{hp}")
                      for hp in range(2)] for d in range(2)]
            # h strips: fp32 + bf16 shadow, written per-step, flushed per half
            hst = [[work.tile([P, U, NW], F32, tag=f"hs{d}h{hp}",
                              name=f"hs{d}h{hp}")
                    for hp in range(2)] for d in range(2)]
            hbf = [[work.tile([P, U, NW], BF16, tag=f"hb{d}h{hp}",
                              name=f"hb{d}h{hp}")
                    for hp in range(2)] for d in range(2)]
            trz = [work.tile([P, RZ], F32, tag=f"trz{d}", name=f"trz{d}")
                   for d in range(2)]
            tn = [work.tile([P, NW], F32, tag=f"tn{d}", name=f"tn{d}")
                  for d in range(2)]
            tn2 = [work.tile([P, NW], F32, tag=f"tn2{d}", name=f"tn2{d}")
                   for d in range(2)]
            psr = [[psum_rec.tile([P, CB], F32, tag=f"ps{d}p{par}",
                                  name=f"ps{d}p{par}")
                    for par in range(2)] for d in range(2)]

            for d in range(2):
                # initial h (=0) lives in strip[1] at the slot the first
                # step reads: fwd reads [1][U-1], bwd reads [1][0]
                nc.vector.memset(hst[d][1][:], 0.0)
                nc.vector.memset(hbf[d][1][:], 0.0)
                # prologue xg prefetch for half 0 of iteration 0
                row0 = U if d == 0 else S
                nc.sync.dma_start(
                    xg_sb[d][0][:],
                    xg_dram[l][d][row0:row0 + U].rearrange("s p c -> p s c"))

            with tc.For_i(0, NIT, 1,
                          hint_engines=(mybir.EngineType.PE,
                                        mybir.EngineType.DVE,
                                        mybir.EngineType.Activation,
                                        mybir.EngineType.SP)) as it:
                g00 = it * (2 * U)
                for half in range(2):
                    hp = half
                    for d in range(2):
                        # prefetch next half's xg into the other buffer
                        if d == 0:
                            row_n = g00 + (2 + half) * U
                        else:
                            row_n = (S - (1 + half) * U) - g00
                        dyn_dma(nc,
                            xg_sb[d][1 - hp][:],
                            xg_dram[l][d][ds(row_n, U), :, :]
                            .rearrange("s p c -> p s c"))

                    for u in range(U):
                        par = u % 2
                        for d in range(2):
                            # strip slots (static):
                            w_slot = u if d == 0 else U - 1 - u
                            if u == 0:
                                rt_t, r_slot = hst[d][1 - hp], (U - 1 if d == 0 else 0)
                                rb_t = hbf[d][1 - hp]
                            else:
                                rt_t = hst[d][hp]
                                rb_t = hbf[d][hp]
                                r_slot = u - 1 if d == 0 else U - u
                            xslot = u if d == 0 else U - 1 - u
                            ps = psr[d][par]
                            xg = xg_sb[d][hp][:, xslot, :]
                            hprev = rt_t[:, r_slot, :]
                            hbprev = rb_t[:, r_slot, :]
                            hnew = hst[d][hp][:, w_slot, :]
                            hbnew = hbf[d][hp][:, w_slot, :]

                            for m in range(MC_N):
                                for k in range(HC_N):
                                    nc.tensor.matmul(
                                        ps[:, m * BL:(m + 1) * BL],
                                        whh_sb[d][:, k, ts(m, P)],
                                        hbprev[:, ts(k, BL)],
                                        start=(k == 0), stop=(k == HC_N - 1))

                            nc.vector.tensor_add(
                                trz[d][:], ps[:, :RZ], xg[:, :RZ])
                            nc.scalar.activation(
                                trz[d][:], trz[d][:], AF.Sigmoid)
                            nc.vector.tensor_add(
                                tn[d][:], ps[:, RZ:CB], bhnb_sb[d][:])
                            nc.vector.tensor_mul(
                                tn[d][:], trz[d][:, :NW], tn[d][:])
                            nc.vector.tensor_add(
                                tn[d][:], tn[d][:], xg[:, RZ:CB])
                            nc.scalar.activation(tn[d][:], tn[d][:], AF.Tanh)
                            nc.vector.tensor_sub(
                                tn2[d][:], hprev[:], tn[d][:])
                            nc.vector.tensor_mul(
                                tn2[d][:], trz[d][:, NW:RZ], tn2[d][:])
                            nc.vector.tensor_add(hnew[:], tn[d][:], tn2[d][:])
                            nc.vector.tensor_copy(hbnew[:], hnew[:])

                    # flush h strips to DRAM (one DMA per dir per half)
                    for d in range(2):
                        dst = (out_f if d == 0 else out_b) if last \
                            else h_hist[l][d]
                        if d == 0:
                            roww = g00 + half * U
                        else:
                            roww = (S - (1 + half) * U) - g00
                        dyn_dma(nc,
                            dst[ds(roww, U), :, :].rearrange("s p c -> p s c"),
                            hst[d][hp][:])
    nc.compile()
    return nc


# ---------------------------------------------------------------------------
# Host side
# ---------------------------------------------------------------------------

def _patchify(x, p=16):
    b, c, W, Hh = x.shape
    w, h = W // p, Hh // p
    x = x.reshape(b, c, w, p, h, p)
    x = x.transpose(0, 2, 4, 3, 5, 1)
    return x.reshape(b, w * h, p * p * c)


def pack_weights(W_ih, W_hh, b_ih, b_hh, BL):
    wihT = np.ascontiguousarray(
        np.asarray(W_ih, np.float32).transpose(0, 1, 3, 2))
    whhT = np.ascontiguousarray(
        np.asarray(W_hh, np.float32).transpose(0, 1, 3, 2)).astype(
            ml_dtypes.bfloat16)
    b_ih = np.asarray(b_ih, np.float32)
    b_hh = np.asarray(b_hh, np.float32)
    brz = b_ih[:, :, :2 * H] + b_hh[:, :, :2 * H]
    bn_i = b_ih[:, :, 2 * H:]
    bias9 = np.concatenate(
        [brz.reshape(DEPTH, 2, 6, P), bn_i.reshape(DEPTH, 2, 3, P)], axis=2)
    bias9 = np.ascontiguousarray(bias9.transpose(0, 1, 3, 2))  # [l,d,p,m]
    bhn = b_hh[:, :, 2 * H:].reshape(DEPTH, 2, 3, 1, P)
    bhnb = np.broadcast_to(bhn, (DEPTH, 2, 3, BL, P)).reshape(
        DEPTH, 2, 3 * BL, P)
    bhnb = np.ascontiguousarray(bhnb.transpose(0, 1, 3, 2))    # [l,d,p,c*BL+b]
    return wihT, whhT, bias9, bhnb


def host_inputs(x, W_ih, W_hh, b_ih, b_hh, n_cores=N_CORES):
    patches = _patchify(np.asarray(x, np.float32))      # (B, S, D)
    B = patches.shape[0]
    BL = B // n_cores
    wihT, whhT, bias9, bhnb = pack_weights(W_ih, W_hh, b_ih, b_hh, BL)
    in_maps = []
    for cc in range(n_cores):
        pt = patches[cc * BL:(cc + 1) * BL]             # (BL, S, D)
        ptT = np.ascontiguousarray(pt.transpose(2, 1, 0)).reshape(D, -1)
        in_maps.append({
            "patchesT": ptT,
            "wihT": wihT,
            "whhT": whhT,
            "bias9": bias9,
            "bhnb": bhnb,
        })
    return in_maps


def unpack_out(res, S, BL):
    """Device out layout [s, p, c*BL+b] -> (BL, S, H)."""
    o = np.asarray(res).reshape(S, P, HC_N, BL)
    return np.ascontiguousarray(o.transpose(3, 0, 2, 1)).reshape(BL, S, H)


_CACHE = {}


def make_jit_runner(nc, n_cores=N_CORES):
    """Persistent jitted SPMD runner (the stock run_bass_via_pjrt re-traces
    and recompiles on every call).  Returns (runner, stage, execute):
    runner(in_maps) -> list of per-core output dicts."""
    import jax
    import numpy as np
    from jax.sharding import Mesh, PartitionSpec, NamedSharding
    from jax.experimental.shard_map import shard_map
    import concourse.mybir as mybir
    from concourse import bass2jax

    bass2jax.install_neuronx_cc_hook()
    assert nc.partition_id_tensor is None or True

    part_name = nc.partition_id_tensor.name if nc.partition_id_tensor else None
    in_names, out_names, out_avals, zero_outs = [], [], [], []
    for alloc in nc.m.functions[0].allocations:
        if not isinstance(alloc, mybir.MemoryLocationSet):
            continue
        name = alloc.memorylocations[0].name
        if alloc.kind == "ExternalInput":
            if name != part_name:
                in_names.append(name)
        elif alloc.kind == "ExternalOutput":
            out_names.append(name)
            shape = tuple(alloc.tensor_shape)
            dtype = mybir.dt.np(alloc.dtype)
            out_avals.append(jax.core.ShapedArray(shape, dtype))
            zero_outs.append(np.zeros(shape, dtype))
    n_params = len(in_names)
    all_names = in_names + out_names
    if part_name is not None:
        all_names = all_names + [part_name]
    donate = tuple(range(n_params, n_params + len(out_names)))

    def _body(*args):
        operands = list(args)
        if part_name is not None:
            operands.append(bass2jax.partition_id_tensor())
        outs = bass2jax._bass_exec_p.bind(
            *operands,
            out_avals=tuple(out_avals),
            in_names=tuple(all_names),
            out_names=tuple(out_names),
            lowering_input_output_aliases=(),
            sim_require_finite=True,
            sim_require_nnan=True,
            nc=nc,
        )
        return tuple(outs)

    devices = jax.devices()[:n_cores]
    mesh = Mesh(np.asarray(devices), ("core",))
    spec = NamedSharding(mesh, PartitionSpec("core"))
    n_all = n_params + len(out_names)
    fn = jax.jit(
        shard_map(_body, mesh=mesh,
                  in_specs=(PartitionSpec("core"),) * n_all,
                  out_specs=(PartitionSpec("core"),) * len(out_names),
                  check_rep=False),
        donate_argnums=donate, keep_unused=True)

    def stage_inputs(in_maps):
        concat = [np.concatenate([np.asarray(m[name]) for m in in_maps], axis=0)
                  for name in in_names]
        return [jax.device_put(a, spec) for a in concat]

    def stage_zeros():
        return [jax.device_put(
            np.zeros((n_cores * z.shape[0], *z.shape[1:]), z.dtype), spec)
            for z in zero_outs]

    def execute(staged_in, staged_zeros):
        return fn(*staged_in, *staged_zeros)

    def runner(in_maps):
        out_arrs = execute(stage_inputs(in_maps), stage_zeros())
        res = []
        for c in range(n_cores):
            res.append({
                name: np.asarray(out_arrs[i]).reshape(
                    n_cores, *out_avals[i].shape)[c]
                for i, name in enumerate(out_names)})
        return res

    runner.stage_inputs = stage_inputs
    runner.stage_zeros = stage_zeros
    runner.execute = execute
    return runner


def get_runner():
    """Build + compile once; returns a callable over in_maps."""
    if "runner" not in _CACHE:
        nc = build_kernel()
        _CACHE["nc"] = nc
        _CACHE["runner"] = make_jit_runner(nc)
    return _CACHE["runner"]


def kernel(x, W_ih, W_hh, b_ih, b_hh):
    in_maps = host_inputs(x, W_ih, W_hh, b_ih, b_hh)
    results = get_runner()(in_maps)
    BL = B_FULL // N_CORES
    y = np.empty((B_FULL, S_FULL, 2 * H), np.float32)
    for cc in range(N_CORES):
        y[cc * BL:(cc + 1) * BL, :, :H] = unpack_out(
            results[cc]["out_f"], S_FULL, BL)
        y[cc * BL:(cc + 1) * BL, :, H:] = unpack_out(
            results[cc]["out_b"], S_FULL, BL)
    return y


# revision 18
# speedup vs baseline: 395.5975x; 21.1510x over previous
"""Bass/Trainium2 kernel for nn_BiGRU_1486058685008.

Reference computation: patchify(x, 16) -> 3-layer bidirectional GRU
(PyTorch gate order r,z,n), input (16, 3, 512, 512) -> (16, 1024, 768).

Strategy: data-parallel over batch across the 8 NeuronCores (B_local=2 per
core); each core runs both directions and all 3 layers locally so no
cross-core communication is needed.  Within a core:

 - Input-gate projections xg = x @ W_ih^T + biases are big fp32 matmuls
   (W_ih^T stationary tiles, tokens moving), evacuated with a per-partition
   bias add, assembled into per-step blocks in SBUF, and staged to DRAM in
   partition-major [P, S, 9*BL] layout so every DMA moves multi-KB
   contiguous runs (small-run scatter DMAs are descriptor-rate-limited).
 - The recurrence keeps gates TRANSPOSED: gates^T (1152 gate dims over 9
   partition-chunks, B_local free).  Stationary = W_hh^T tiles in bf16 (FWL
   double-rate weight loads), moving = h^T bf16 (tiny N).  PSUM holds
   (128, 9*BL) per step; elementwise runs on short (128, *) tiles.
   h is kept fp32; a bf16 shadow feeds the next matmul.
 - The For_i body covers 2*UNROLL steps (two halves).  All per-step
   addressing is static: xg is fetched one half (UNROLL steps) at a time
   with a single dynamic-offset DMA into a double-buffered SBUF block, and
   h is written into an SBUF strip that is flushed to DRAM once per half.
   This keeps dynamic-AP register pressure tiny (the per-engine register
   files cannot hold per-step dynamic offsets for an unrolled body).
 - Backward-direction data uses natural time order everywhere; the bwd
   half simply addresses its strip/ring slots reversed (statically).
"""

import contextlib

import numpy as np
import ml_dtypes

import concourse.bass as bass
import concourse.mybir as mybir
import concourse.tile as tile
from concourse import bacc
from concourse.bass import ds, ts

F32 = mybir.dt.float32
BF16 = mybir.dt.bfloat16
AF = mybir.ActivationFunctionType

P = 128
D = 768         # input dim of every layer (patch dim; 2H for layers 1,2)
H = 384         # hidden per direction
G3 = 3 * H      # 1152
KC_N = D // P   # 6 contraction chunks for projections
MC_N = G3 // P  # 9 gate chunks
HC_N = H // P   # 3 hidden chunks
DEPTH = 3
S_FULL = 1024
B_FULL = 16
N_CORES = 8



# ---------------------------------------------------------------------------
# Kernel builder
# ---------------------------------------------------------------------------

def build_kernel(S=S_FULL, BL=B_FULL // N_CORES, U=32, reps=1, phases="all"):
    """Build the per-core Bass program.

    S: sequence length, BL: local batch, U: steps per half-body (the For_i
    body covers 2*U steps).  xg DRAM rows are shifted by U: time s lives at
    row s+U, so both directions' half-block prefetch stays in bounds.
    """
    assert S % (2 * U) == 0
    T = S * BL
    CB = MC_N * BL          # columns per xg step block (9*BL)
    NW = 3 * BL             # one gate region width
    RZ = 6 * BL             # r+z region width
    TOK = 512 if T % 512 == 0 else T // 2
    assert T % TOK == 0 and TOK % BL == 0
    TB_N = T // TOK
    S_TOK = TOK // BL       # s-steps covered by one token block
    NIT = S // (2 * U)

    nc = bacc.Bacc()

    patchesT = nc.declare_dram_parameter("patchesT", [D, T], F32, isOutput=False)
    wihT = nc.declare_dram_parameter("wihT", [DEPTH, 2, D, G3], F32, isOutput=False)
    whhT = nc.declare_dram_parameter("whhT", [DEPTH, 2, H, G3], BF16, isOutput=False)
    # bias9[l,d,p,m]: m<6 -> (b_ih+b_hh) for r,z chunks; m>=6 -> b_ih n chunks
    bias9 = nc.declare_dram_parameter("bias9", [DEPTH, 2, P, MC_N], F32, isOutput=False)
    # bhnb[l,d,p,c*BL+b] = b_hh n-gate, broadcast over local batch
    bhnb = nc.declare_dram_parameter("bhnb", [DEPTH, 2, P, NW], F32, isOutput=False)

    # outputs in device layout [p, s, c*BL+b]; host reassembles
    out_f = nc.declare_dram_parameter("out_f", [P, S, NW], F32, isOutput=True)
    out_b = nc.declare_dram_parameter("out_b", [P, S, NW], F32, isOutput=True)

    # Dynamic-offset DMAs consume per-engine registers that are never
    # recycled (~4-5 each, 54 per engine); round-robin them across engine
    # queues so no single engine's register file is exhausted.
    _dyn = {"i": 0}

    def dyn_dma(nc, out, in_):
        engs = [nc.sync, nc.scalar, nc.gpsimd]
        e = engs[_dyn["i"] % len(engs)]
        _dyn["i"] += 1
        e.dma_start(out, in_)

    with tile.TileContext(nc) as tc, contextlib.ExitStack() as ctx:
        consts = ctx.enter_context(tc.tile_pool(name="consts", bufs=1))
        wih_pool = ctx.enter_context(tc.tile_pool(name="wih", bufs=6))
        rhs_pool = ctx.enter_context(tc.tile_pool(name="rhs", bufs=1))
        stage_pool = ctx.enter_context(tc.tile_pool(name="stage", bufs=3))
        psum_proj = ctx.enter_context(
            tc.tile_pool(name="psum_proj", bufs=4, space="PSUM"))
        psum_rec = ctx.enter_context(
            tc.tile_pool(name="psum_rec", bufs=1, space="PSUM"))
        dram = ctx.enter_context(tc.tile_pool(name="dram", bufs=1, space="DRAM"))
        work = ctx.enter_context(tc.tile_pool(name="work", bufs=1))

        # persistent DRAM scratch
        xg_dram = [[dram.tile([P, S + 2 * U, CB], F32,
                              tag=f"xg{l}d{d}", name=f"xg{l}d{d}")
                    for d in range(2)] for l in range(DEPTH)]
        h_hist = [[dram.tile([P, S, NW], F32, tag=f"hh{l}d{d}",
                             name=f"hh{l}d{d}")
                   for d in range(2)] for l in range(2)]

        # zero-fill the xg slack rows (prefetch overruns read them; the
        # values are never consumed but must not be NaN for the simulator)
        zf = work.tile([P, U, CB], F32, tag="zf", name="zf")
        nc.vector.memset(zf[:], 0.0)
        for l in range(DEPTH):
            for d in range(2):
                for r0 in (0, S + U):
                    nc.sync.dma_start(
                        xg_dram[l][d][:, r0:r0 + U, :], zf[:])

        for rep in range(reps):
          for l in range(DEPTH):
            last = (l == DEPTH - 1)

            # ---------------- per-layer constants ----------------
            whh_sb, bias9_sb, bhnb_sb = [], [], []
            for d in range(2):
                w = consts.tile([P, HC_N, G3], BF16, tag=f"whh{d}",
                                name=f"whh{d}")
                nc.sync.dma_start(
                    w[:], whhT[l, d].rearrange("(k p) g -> p k g", p=P))
                whh_sb.append(w)
                b9 = consts.tile([P, MC_N], F32, tag=f"b9{d}", name=f"b9{d}")
                nc.sync.dma_start(b9[:], bias9[l, d])
                bias9_sb.append(b9)
                bh = consts.tile([P, NW], F32, tag=f"bh{d}", name=f"bh{d}")
                nc.sync.dma_start(bh[:], bhnb[l, d])
                bhnb_sb.append(bh)

            # ---------------- projection phase ----------------
            # rhs: patchesT tiles (l=0) or staged h-history tiles (l>0);
            # staged tiles slice directly as the matmul moving operand.
            skip_proj = (phases == "rec")
            rhs_aps = {}
            for tb in range(TB_N if not skip_proj else 0):
                if l == 0:
                    for kc in range(KC_N):
                        rt = rhs_pool.tile([P, TOK], F32, tag=f"rhs{kc}_{tb}",
                                           name=f"rhs{kc}_{tb}")
                        nc.sync.dma_start(
                            rt[:], patchesT[ts(kc, P), ts(tb, TOK)])
                        rhs_aps[(kc, tb)] = rt[:].rearrange(
                            "p (s b) -> p s b", b=BL)
                else:
                    s0 = tb * S_TOK
                    for dh in range(2):
                        ht = rhs_pool.tile([P, S_TOK, NW], F32,
                                           tag=f"hst{dh}_{tb}",
                                           name=f"hst{dh}_{tb}")
                        nc.sync.dma_start(
                            ht[:], h_hist[l - 1][dh][:, s0:s0 + S_TOK, :])
                        for c in range(HC_N):
                            rhs_aps[(dh * HC_N + c, tb)] = \
                                ht[:, :, ts(c, BL)]

            for d in range(2 if not skip_proj else 0):
                wsb = consts.tile([P, KC_N, G3], F32, tag="wih", name="wih")
                nc.sync.dma_start(
                    wsb[:], wihT[l, d].rearrange("(k p) g -> p k g", p=P))
                for tb in range(TB_N):
                    stb = stage_pool.tile([P, S_TOK, CB], F32, tag="stb",
                                          name="stb")
                    for m in range(MC_N):
                        ps = psum_proj.tile([P, TOK], F32, name="psp")
                        for kc in range(KC_N):
                            nc.tensor.matmul(
                                ps[:].rearrange("p (s b) -> p s b", b=BL),
                                wsb[:, kc, ts(m, P)],
                                rhs_aps[(kc, tb)],
                                start=(kc == 0), stop=(kc == KC_N - 1))
                        nc.vector.tensor_scalar_add(
                            stb[:, :, m * BL:(m + 1) * BL],
                            ps[:].rearrange("p (s b) -> p s b", b=BL),
                            bias9_sb[d][:, m:m + 1])
                    s0 = tb * S_TOK + U
                    nc.sync.dma_start(
                        xg_dram[l][d][:, s0:s0 + S_TOK, :], stb[:])

            # ---------------- recurrence phase ----------------
            if phases == "proj":
                continue
            # xg ring: one half (U steps) per buffer, double-buffered
            xg_sb = [[work.tile([P, U, CB], F32, tag=f"xg{d}h{hp}",
                                name=f"xg{d}h{hp}")
                      for hp in range(2)] for d in range(2)]
            # h strips: fp32 + bf16 shadow, written per-step, flushed per half
            hst = [[work.tile([P, U, NW], F32, tag=f"hs{d}h{hp}",
                              name=f"hs{d}h{hp}")
                    for hp in range(2)] for d in range(2)]
            hbf = [[work.tile([P, U, NW], BF16, tag=f"hb{d}h{hp}",
                              name=f"hb{d}h{hp}")
                    for hp in range(2)] for d in range(2)]
            trz = [work.tile([P, RZ], F32, tag=f"trz{d}", name=f"trz{d}")
                   for d in range(2)]
            tn = [work.tile([P, NW], F32, tag=f"tn{d}", name=f"tn{d}")
                  for d in range(2)]
            tn2 = [work.tile([P, NW], F32, tag=f"tn2{d}", name=f"tn2{d}")
                   for d in range(2)]
            psr = [[psum_rec.tile([P, CB], F32, tag=f"ps{d}p{par}",
                                  name=f"ps{d}p{par}")
                    for par in range(2)] for d in range(2)]

            for d in range(2):
                # initial h (=0) lives in strip[1] at the slot the first
                # step reads: fwd reads [1][U-1], bwd reads [1][0]
                nc.vector.memset(hst[d][1][:], 0.0)
                nc.vector.memset(hbf[d][1][:], 0.0)
                # prologue xg prefetch for half 0 of iteration 0
                row0 = U if d == 0 else S
                nc.sync.dma_start(
                    xg_sb[d][0][:], xg_dram[l][d][:, row0:row0 + U, :])

            with tc.For_i(0, NIT, 1,
                          hint_engines=(mybir.EngineType.PE,
                                        mybir.EngineType.DVE,
                                        mybir.EngineType.Activation,
                                        mybir.EngineType.SP)) as it:
                g00 = it * (2 * U)
                for half in range(2):
                    hp = half
                    for d in range(2):
                        # prefetch next half's xg into the other buffer
                        if d == 0:
                            row_n = g00 + (2 + half) * U
                        else:
                            row_n = (S - (1 + half) * U) - g00
                        dyn_dma(nc,
                            xg_sb[d][1 - hp][:],
                            xg_dram[l][d][:, ds(row_n, U), :])

                    for u in range(U):
                        par = u % 2
                        for d in range(2):
                            # strip slots (static):
                            w_slot = u if d == 0 else U - 1 - u
                            if u == 0:
                                rt_t, r_slot = hst[d][1 - hp], (U - 1 if d == 0 else 0)
                                rb_t = hbf[d][1 - hp]
                            else:
                                rt_t = hst[d][hp]
                                rb_t = hbf[d][hp]
                                r_slot = u - 1 if d == 0 else U - u
                            xslot = u if d == 0 else U - 1 - u
                            ps = psr[d][par]
                            xg = xg_sb[d][hp][:, xslot, :]
                            hprev = rt_t[:, r_slot, :]
                            hbprev = rb_t[:, r_slot, :]
                            hnew = hst[d][hp][:, w_slot, :]
                            hbnew = hbf[d][hp][:, w_slot, :]

                            for m in range(MC_N):
                                for k in range(HC_N):
                                    nc.tensor.matmul(
                                        ps[:, m * BL:(m + 1) * BL],
                                        whh_sb[d][:, k, ts(m, P)],
                                        hbprev[:, ts(k, BL)],
                                        start=(k == 0), stop=(k == HC_N - 1))

                            nc.vector.tensor_add(
                                trz[d][:], ps[:, :RZ], xg[:, :RZ])
                            nc.scalar.activation(
                                trz[d][:], trz[d][:], AF.Sigmoid)
                            nc.vector.tensor_add(
                                tn[d][:], ps[:, RZ:CB], bhnb_sb[d][:])
                            nc.vector.tensor_mul(
                                tn[d][:], trz[d][:, :NW], tn[d][:])
                            nc.vector.tensor_add(
                                tn[d][:], tn[d][:], xg[:, RZ:CB])
                            # tanh(x) = 2*sigmoid(2x) - 1: keeps every ACT op
                            # in the sigmoid table set (a tanh op would pull a
                            # different act-function set and force a ~2.7us
                            # table reload on every sigmoid<->tanh switch)
                            nc.scalar.activation(tn[d][:], tn[d][:],
                                                 AF.Sigmoid, scale=2.0)
                            nc.vector.tensor_scalar(
                                tn[d][:], tn[d][:], 2.0, -1.0,
                                mybir.AluOpType.mult, mybir.AluOpType.add)
                            nc.vector.tensor_sub(
                                tn2[d][:], hprev[:], tn[d][:])
                            nc.vector.tensor_mul(
                                tn2[d][:], trz[d][:, NW:RZ], tn2[d][:])
                            nc.vector.tensor_add(hnew[:], tn[d][:], tn2[d][:])
                            nc.vector.tensor_copy(hbnew[:], hnew[:])

                    # flush h strips to DRAM (one DMA per dir per half)
                    for d in range(2):
                        dst = (out_f if d == 0 else out_b) if last \
                            else h_hist[l][d]
                        if d == 0:
                            roww = g00 + half * U
                        else:
                            roww = (S - (1 + half) * U) - g00
                        dyn_dma(nc,
                            dst[:, ds(roww, U), :], hst[d][hp][:])
    nc.compile()
    return nc


# ---------------------------------------------------------------------------
# Host side
# ---------------------------------------------------------------------------

def _patchify(x, p=16):
    b, c, W, Hh = x.shape
    w, h = W // p, Hh // p
    x = x.reshape(b, c, w, p, h, p)
    x = x.transpose(0, 2, 4, 3, 5, 1)
    return x.reshape(b, w * h, p * p * c)


def pack_weights(W_ih, W_hh, b_ih, b_hh, BL):
    wihT = np.ascontiguousarray(
        np.asarray(W_ih, np.float32).transpose(0, 1, 3, 2))
    whhT = np.ascontiguousarray(
        np.asarray(W_hh, np.float32).transpose(0, 1, 3, 2)).astype(
            ml_dtypes.bfloat16)
    b_ih = np.asarray(b_ih, np.float32)
    b_hh = np.asarray(b_hh, np.float32)
    brz = b_ih[:, :, :2 * H] + b_hh[:, :, :2 * H]
    bn_i = b_ih[:, :, 2 * H:]
    bias9 = np.concatenate(
        [brz.reshape(DEPTH, 2, 6, P), bn_i.reshape(DEPTH, 2, 3, P)], axis=2)
    bias9 = np.ascontiguousarray(bias9.transpose(0, 1, 3, 2))  # [l,d,p,m]
    bhn = b_hh[:, :, 2 * H:].reshape(DEPTH, 2, 3, 1, P)
    bhnb = np.broadcast_to(bhn, (DEPTH, 2, 3, BL, P)).reshape(
        DEPTH, 2, 3 * BL, P)
    bhnb = np.ascontiguousarray(bhnb.transpose(0, 1, 3, 2))    # [l,d,p,c*BL+b]
    return wihT, whhT, bias9, bhnb


def host_inputs(x, W_ih, W_hh, b_ih, b_hh, n_cores=N_CORES):
    patches = _patchify(np.asarray(x, np.float32))      # (B, S, D)
    B = patches.shape[0]
    BL = B // n_cores
    wihT, whhT, bias9, bhnb = pack_weights(W_ih, W_hh, b_ih, b_hh, BL)
    in_maps = []
    for cc in range(n_cores):
        pt = patches[cc * BL:(cc + 1) * BL]             # (BL, S, D)
        ptT = np.ascontiguousarray(pt.transpose(2, 1, 0)).reshape(D, -1)
        in_maps.append({
            "patchesT": ptT,
            "wihT": wihT,
            "whhT": whhT,
            "bias9": bias9,
            "bhnb": bhnb,
        })
    return in_maps


def unpack_out(res, S, BL):
    """Device out layout [p, s, c*BL+b] -> (BL, S, H)."""
    o = np.asarray(res).reshape(P, S, HC_N, BL)
    return np.ascontiguousarray(o.transpose(3, 1, 2, 0)).reshape(BL, S, H)


_CACHE = {}


def make_jit_runner(nc, n_cores=N_CORES):
    """Persistent jitted SPMD runner (the stock run_bass_via_pjrt re-traces
    and recompiles on every call).  Returns (runner, stage, execute):
    runner(in_maps) -> list of per-core output dicts."""
    import jax
    import numpy as np
    from jax.sharding import Mesh, PartitionSpec, NamedSharding
    from jax.experimental.shard_map import shard_map
    import concourse.mybir as mybir
    from concourse import bass2jax

    bass2jax.install_neuronx_cc_hook()
    assert nc.partition_id_tensor is None or True

    part_name = nc.partition_id_tensor.name if nc.partition_id_tensor else None
    in_names, out_names, out_avals, zero_outs = [], [], [], []
    for alloc in nc.m.functions[0].allocations:
        if not isinstance(alloc, mybir.MemoryLocationSet):
            continue
        name = alloc.memorylocations[0].name
        if alloc.kind == "ExternalInput":
            if name != part_name:
                in_names.append(name)
        elif alloc.kind == "ExternalOutput":
            out_names.append(name)
            shape = tuple(alloc.tensor_shape)
            dtype = mybir.dt.np(alloc.dtype)
            out_avals.append(jax.core.ShapedArray(shape, dtype))
            zero_outs.append(np.zeros(shape, dtype))
    n_params = len(in_names)
    all_names = in_names + out_names
    if part_name is not None:
        all_names = all_names + [part_name]
    donate = tuple(range(n_params, n_params + len(out_names)))

    def _body(*args):
        operands = list(args)
        if part_name is not None:
            operands.append(bass2jax.partition_id_tensor())
        outs = bass2jax._bass_exec_p.bind(
            *operands,
            out_avals=tuple(out_avals),
            in_names=tuple(all_names),
            out_names=tuple(out_names),
            lowering_input_output_aliases=(),
            sim_require_finite=True,
            sim_require_nnan=True,
            nc=nc,
        )
        return tuple(outs)

    devices = jax.devices()[:n_cores]
    mesh = Mesh(np.asarray(devices), ("core",))
    spec = NamedSharding(mesh, PartitionSpec("core"))
    n_all = n_params + len(out_names)
    fn = jax.jit(
        shard_map(_body, mesh=mesh,
                  in_specs=(PartitionSpec("core"),) * n_all,
                  out_specs=(PartitionSpec("core"),) * len(out_names),
                  check_rep=False),
        donate_argnums=donate, keep_unused=True)

    def stage_inputs(in_maps):
        concat = [np.concatenate([np.asarray(m[name]) for m in in_maps], axis=0)
                  for name in in_names]
        return [jax.device_put(a, spec) for a in concat]

    def stage_zeros():
        return [jax.device_put(
            np.zeros((n_cores * z.shape[0], *z.shape[1:]), z.dtype), spec)
            for z in zero_outs]

    def execute(staged_in, staged_zeros):
        return fn(*staged_in, *staged_zeros)

    def runner(in_maps):
        out_arrs = execute(stage_inputs(in_maps), stage_zeros())
        res = []
        for c in range(n_cores):
            res.append({
                name: np.asarray(out_arrs[i]).reshape(
                    n_cores, *out_avals[i].shape)[c]
                for i, name in enumerate(out_names)})
        return res

    runner.stage_inputs = stage_inputs
    runner.stage_zeros = stage_zeros
    runner.execute = execute
    return runner


def get_runner():
    """Build + compile once; returns a callable over in_maps."""
    if "runner" not in _CACHE:
        nc = build_kernel()
        _CACHE["nc"] = nc
        _CACHE["runner"] = make_jit_runner(nc)
    return _CACHE["runner"]


def kernel(x, W_ih, W_hh, b_ih, b_hh):
    in_maps = host_inputs(x, W_ih, W_hh, b_ih, b_hh)
    results = get_runner()(in_maps)
    BL = B_FULL // N_CORES
    y = np.empty((B_FULL, S_FULL, 2 * H), np.float32)
    for cc in range(N_CORES):
        y[cc * BL:(cc + 1) * BL, :, :H] = unpack_out(
            results[cc]["out_f"], S_FULL, BL)
        y[cc * BL:(cc + 1) * BL, :, H:] = unpack_out(
            results[cc]["out_b"], S_FULL, BL)
    return y


# revision 20
# speedup vs baseline: 503.6374x; 1.2731x over previous
"""Bass/Trainium2 kernel for nn_BiGRU_1486058685008.

Reference computation: patchify(x, 16) -> 3-layer bidirectional GRU
(PyTorch gate order r,z,n), input (16, 3, 512, 512) -> (16, 1024, 768).

Strategy: data-parallel over batch across the 8 NeuronCores (B_local=2 per
core); each core runs both directions and all 3 layers locally so no
cross-core communication is needed.  Within a core:

 - Input-gate projections xg = x @ W_ih^T + biases are big fp32 matmuls
   (W_ih^T stationary tiles, tokens moving), evacuated with a per-partition
   bias add, assembled into per-step blocks in SBUF, and staged to DRAM in
   partition-major [P, S, 9*BL] layout so every DMA moves multi-KB
   contiguous runs (small-run scatter DMAs are descriptor-rate-limited).
 - The recurrence keeps gates TRANSPOSED: gates^T (1152 gate dims over 9
   partition-chunks, B_local free).  Stationary = W_hh^T tiles in bf16 (FWL
   double-rate weight loads), moving = h^T bf16 (tiny N).  PSUM holds
   (128, 9*BL) per step; elementwise runs on short (128, *) tiles.
   h is kept fp32; a bf16 shadow feeds the next matmul.
 - The For_i body covers 2*UNROLL steps (two halves).  All per-step
   addressing is static: xg is fetched one half (UNROLL steps) at a time
   with a single dynamic-offset DMA into a double-buffered SBUF block, and
   h is written into an SBUF strip that is flushed to DRAM once per half.
   This keeps dynamic-AP register pressure tiny (the per-engine register
   files cannot hold per-step dynamic offsets for an unrolled body).
 - Backward-direction data uses natural time order everywhere; the bwd
   half simply addresses its strip/ring slots reversed (statically).
"""

import contextlib

import numpy as np
import ml_dtypes

import concourse.bass as bass
import concourse.mybir as mybir
import concourse.tile as tile
from concourse import bacc
from concourse.bass import ds, ts

F32 = mybir.dt.float32
BF16 = mybir.dt.bfloat16
AF = mybir.ActivationFunctionType

P = 128
D = 768         # input dim of every layer (patch dim; 2H for layers 1,2)
H = 384         # hidden per direction
G3 = 3 * H      # 1152
KC_N = D // P   # 6 contraction chunks for projections
MC_N = G3 // P  # 9 gate chunks
HC_N = H // P   # 3 hidden chunks
DEPTH = 3
S_FULL = 1024
B_FULL = 16
N_CORES = 8



# ---------------------------------------------------------------------------
# Kernel builder
# ---------------------------------------------------------------------------

def build_kernel(S=S_FULL, BL=B_FULL // N_CORES, U=32, reps=1, phases="all"):
    """Build the per-core Bass program.

    S: sequence length, BL: local batch, U: steps per half-body (the For_i
    body covers 2*U steps).  xg DRAM rows are shifted by U: time s lives at
    row s+U, so both directions' half-block prefetch stays in bounds.
    """
    assert S % (2 * U) == 0
    T = S * BL
    CB = MC_N * BL          # columns per xg step block (9*BL)
    NW = 3 * BL             # one gate region width
    RZ = 6 * BL             # r+z region width
    TOK = 512 if T % 512 == 0 else T // 2
    assert T % TOK == 0 and TOK % BL == 0
    TB_N = T // TOK
    S_TOK = TOK // BL       # s-steps covered by one token block
    NIT = S // (2 * U)

    nc = bacc.Bacc()

    patchesT = nc.declare_dram_parameter("patchesT", [D, T], F32, isOutput=False)
    wihT = nc.declare_dram_parameter("wihT", [DEPTH, 2, D, G3], F32, isOutput=False)
    whhT = nc.declare_dram_parameter("whhT", [DEPTH, 2, H, G3], BF16, isOutput=False)
    # bias9[l,d,p,m]: m<6 -> (b_ih+b_hh) for r,z chunks; m>=6 -> b_ih n chunks
    bias9 = nc.declare_dram_parameter("bias9", [DEPTH, 2, P, MC_N], F32, isOutput=False)
    # bhnb[l,d,p,c*BL+b] = b_hh n-gate, broadcast over local batch
    bhnb = nc.declare_dram_parameter("bhnb", [DEPTH, 2, P, NW], F32, isOutput=False)

    # outputs in device layout [p, s, c*BL+b]; host reassembles
    out_f = nc.declare_dram_parameter("out_f", [P, S, NW], F32, isOutput=True)
    out_b = nc.declare_dram_parameter("out_b", [P, S, NW], F32, isOutput=True)

    # Dynamic-offset DMAs consume per-engine registers that are never
    # recycled (~4-5 each, 54 per engine); round-robin them across engine
    # queues so no single engine's register file is exhausted.
    _dyn = {"i": 0}

    def dyn_dma(nc, out, in_):
        engs = [nc.sync, nc.scalar, nc.gpsimd]
        e = engs[_dyn["i"] % len(engs)]
        _dyn["i"] += 1
        e.dma_start(out, in_)

    with tile.TileContext(nc) as tc, contextlib.ExitStack() as ctx:
        consts = ctx.enter_context(tc.tile_pool(name="consts", bufs=1))
        wih_pool = ctx.enter_context(tc.tile_pool(name="wih", bufs=6))
        rhs_pool = ctx.enter_context(tc.tile_pool(name="rhs", bufs=1))
        stage_pool = ctx.enter_context(tc.tile_pool(name="stage", bufs=3))
        psum_proj = ctx.enter_context(
            tc.tile_pool(name="psum_proj", bufs=4, space="PSUM"))
        psum_rec = ctx.enter_context(
            tc.tile_pool(name="psum_rec", bufs=1, space="PSUM"))
        dram = ctx.enter_context(tc.tile_pool(name="dram", bufs=1, space="DRAM"))
        work = ctx.enter_context(tc.tile_pool(name="work", bufs=1))

        # persistent DRAM scratch
        xg_dram = [[dram.tile([P, S + 2 * U, CB], F32,
                              tag=f"xg{l}d{d}", name=f"xg{l}d{d}")
                    for d in range(2)] for l in range(DEPTH)]
        h_hist = [[dram.tile([P, S, NW], F32, tag=f"hh{l}d{d}",
                             name=f"hh{l}d{d}")
                   for d in range(2)] for l in range(2)]

        # fp32 identity for PSUM prefill matmuls (xg + n-gate bias are
        # injected into the accumulation group via identity matmuls, which
        # removes two PSUM-source DVE ops from the per-step critical chain)
        from concourse.masks import make_identity
        ident = work.tile([P, P], F32, tag="ident", name="ident")
        make_identity(nc, ident[:])

        # zero-fill the xg slack rows (prefetch overruns read them; the
        # values are never consumed but must not be NaN for the simulator)
        zf = work.tile([P, U, CB], F32, tag="zf", name="zf")
        nc.vector.memset(zf[:], 0.0)
        for l in range(DEPTH):
            for d in range(2):
                for r0 in (0, S + U):
                    nc.sync.dma_start(
                        xg_dram[l][d][:, r0:r0 + U, :], zf[:])

        for rep in range(reps):
          for l in range(DEPTH):
            last = (l == DEPTH - 1)

            # ---------------- per-layer constants ----------------
            whh_sb, bias9_sb, bhnb_sb = [], [], []
            for d in range(2):
                w = consts.tile([P, HC_N, G3], BF16, tag=f"whh{d}",
                                name=f"whh{d}")
                nc.sync.dma_start(
                    w[:], whhT[l, d].rearrange("(k p) g -> p k g", p=P))
                whh_sb.append(w)
                b9 = consts.tile([P, MC_N], F32, tag=f"b9{d}", name=f"b9{d}")
                nc.sync.dma_start(b9[:], bias9[l, d])
                bias9_sb.append(b9)
                bh = consts.tile([P, NW], F32, tag=f"bh{d}", name=f"bh{d}")
                nc.sync.dma_start(bh[:], bhnb[l, d])
                bhnb_sb.append(bh)

            # ---------------- projection phase ----------------
            # rhs: patchesT tiles (l=0) or staged h-history tiles (l>0);
            # staged tiles slice directly as the matmul moving operand.
            skip_proj = (phases == "rec")
            rhs_aps = {}
            for tb in range(TB_N if not skip_proj else 0):
                if l == 0:
                    for kc in range(KC_N):
                        rt = rhs_pool.tile([P, TOK], F32, tag=f"rhs{kc}_{tb}",
                                           name=f"rhs{kc}_{tb}")
                        nc.sync.dma_start(
                            rt[:], patchesT[ts(kc, P), ts(tb, TOK)])
                        rhs_aps[(kc, tb)] = rt[:].rearrange(
                            "p (s b) -> p s b", b=BL)
                else:
                    s0 = tb * S_TOK
                    for dh in range(2):
                        ht = rhs_pool.tile([P, S_TOK, NW], F32,
                                           tag=f"hst{dh}_{tb}",
                                           name=f"hst{dh}_{tb}")
                        nc.sync.dma_start(
                            ht[:], h_hist[l - 1][dh][:, s0:s0 + S_TOK, :])
                        for c in range(HC_N):
                            rhs_aps[(dh * HC_N + c, tb)] = \
                                ht[:, :, ts(c, BL)]

            for d in range(2 if not skip_proj else 0):
                wsb = consts.tile([P, KC_N, G3], F32, tag="wih", name="wih")
                nc.sync.dma_start(
                    wsb[:], wihT[l, d].rearrange("(k p) g -> p k g", p=P))
                for tb in range(TB_N):
                    stb = stage_pool.tile([P, S_TOK, CB], F32, tag="stb",
                                          name="stb")
                    for m in range(MC_N):
                        ps = psum_proj.tile([P, TOK], F32, name="psp")
                        for kc in range(KC_N):
                            nc.tensor.matmul(
                                ps[:].rearrange("p (s b) -> p s b", b=BL),
                                wsb[:, kc, ts(m, P)],
                                rhs_aps[(kc, tb)],
                                start=(kc == 0), stop=(kc == KC_N - 1))
                        nc.vector.tensor_scalar_add(
                            stb[:, :, m * BL:(m + 1) * BL],
                            ps[:].rearrange("p (s b) -> p s b", b=BL),
                            bias9_sb[d][:, m:m + 1])
                    s0 = tb * S_TOK + U
                    nc.sync.dma_start(
                        xg_dram[l][d][:, s0:s0 + S_TOK, :], stb[:])

            # ---------------- recurrence phase ----------------
            if phases == "proj":
                continue
            # xg ring: one half (U steps) per buffer, double-buffered
            xg_sb = [[work.tile([P, U, CB], F32, tag=f"xg{d}h{hp}",
                                name=f"xg{d}h{hp}")
                      for hp in range(2)] for d in range(2)]
            # h strips: fp32 + bf16 shadow, written per-step, flushed per half
            hst = [[work.tile([P, U, NW], F32, tag=f"hs{d}h{hp}",
                              name=f"hs{d}h{hp}")
                    for hp in range(2)] for d in range(2)]
            hbf = [[work.tile([P, U, NW], BF16, tag=f"hb{d}h{hp}",
                              name=f"hb{d}h{hp}")
                    for hp in range(2)] for d in range(2)]
            trz = [work.tile([P, RZ], F32, tag=f"trz{d}", name=f"trz{d}")
                   for d in range(2)]
            tn = [work.tile([P, NW], F32, tag=f"tn{d}", name=f"tn{d}")
                  for d in range(2)]
            tn2 = [work.tile([P, NW], F32, tag=f"tn2{d}", name=f"tn2{d}")
                   for d in range(2)]
            psr = [[psum_rec.tile([P, CB], F32, tag=f"ps{d}p{par}",
                                  name=f"ps{d}p{par}")
                    for par in range(2)] for d in range(2)]

            for d in range(2):
                # initial h (=0) lives in strip[1] at the slot the first
                # step reads: fwd reads [1][U-1], bwd reads [1][0]
                nc.vector.memset(hst[d][1][:], 0.0)
                nc.vector.memset(hbf[d][1][:], 0.0)
                # prologue xg prefetch for half 0 of iteration 0
                row0 = U if d == 0 else S
                nc.sync.dma_start(
                    xg_sb[d][0][:], xg_dram[l][d][:, row0:row0 + U, :])

            with tc.For_i(0, NIT, 1,
                          hint_engines=(mybir.EngineType.PE,
                                        mybir.EngineType.DVE,
                                        mybir.EngineType.Activation,
                                        mybir.EngineType.SP)) as it:
                g00 = it * (2 * U)
                for half in range(2):
                    hp = half
                    for d in range(2):
                        # prefetch next half's xg into the other buffer
                        if d == 0:
                            row_n = g00 + (2 + half) * U
                        else:
                            row_n = (S - (1 + half) * U) - g00
                        dyn_dma(nc,
                            xg_sb[d][1 - hp][:],
                            xg_dram[l][d][:, ds(row_n, U), :])

                    for u in range(U):
                        par = u % 2
                        for d in range(2):
                            # strip slots (static):
                            w_slot = u if d == 0 else U - 1 - u
                            if u == 0:
                                rt_t, r_slot = hst[d][1 - hp], (U - 1 if d == 0 else 0)
                                rb_t = hbf[d][1 - hp]
                            else:
                                rt_t = hst[d][hp]
                                rb_t = hbf[d][hp]
                                r_slot = u - 1 if d == 0 else U - u
                            xslot = u if d == 0 else U - 1 - u
                            ps = psr[d][par]
                            xg = xg_sb[d][hp][:, xslot, :]
                            hprev = rt_t[:, r_slot, :]
                            hbprev = rb_t[:, r_slot, :]
                            hnew = hst[d][hp][:, w_slot, :]
                            hbnew = hbf[d][hp][:, w_slot, :]

                            # prefill: ps[:, :RZ] = xg_rz, ps[:, RZ:] = bhn
                            # (start=True clears the bank's has_written bits;
                            # the weight matmuls then accumulate on top)
                            nc.tensor.matmul(
                                ps[:, :RZ], ident[:], xg[:, :RZ],
                                start=True, stop=False, skip_group_check=True)
                            nc.tensor.matmul(
                                ps[:, RZ:CB], ident[:], bhnb_sb[d][:],
                                start=False, stop=False, skip_group_check=True)
                            for m in range(MC_N):
                                for k in range(HC_N):
                                    nc.tensor.matmul(
                                        ps[:, m * BL:(m + 1) * BL],
                                        whh_sb[d][:, k, ts(m, P)],
                                        hbprev[:, ts(k, BL)],
                                        start=False,
                                        stop=(m == MC_N - 1 and
                                              k == HC_N - 1),
                                        skip_group_check=True)

                            nc.scalar.activation(
                                trz[d][:], ps[:, :RZ], AF.Sigmoid)
                            nc.vector.tensor_mul(
                                tn[d][:], trz[d][:, :NW], ps[:, RZ:CB])
                            nc.vector.tensor_add(
                                tn[d][:], tn[d][:], xg[:, RZ:CB])
                            # tanh(x) = 2*sigmoid(2x) - 1: keeps every ACT op
                            # in the sigmoid table set (a tanh op would pull a
                            # different act-function set and force a ~2.7us
                            # table reload on every sigmoid<->tanh switch)
                            nc.scalar.activation(tn[d][:], tn[d][:],
                                                 AF.Sigmoid, scale=2.0)
                            nc.vector.tensor_scalar(
                                tn[d][:], tn[d][:], 2.0, -1.0,
                                mybir.AluOpType.mult, mybir.AluOpType.add)
                            nc.vector.tensor_sub(
                                tn2[d][:], hprev[:], tn[d][:])
                            nc.vector.tensor_mul(
                                tn2[d][:], trz[d][:, NW:RZ], tn2[d][:])
                            # two independent adds: the bf16 shadow (what the
                            # next step's matmuls wait on) no longer serializes
                            # behind the fp32 write + a cast copy
                            nc.vector.tensor_add(hbnew[:], tn[d][:], tn2[d][:])
                            nc.vector.tensor_add(hnew[:], tn[d][:], tn2[d][:])

                    # flush h strips to DRAM (one DMA per dir per half)
                    for d in range(2):
                        dst = (out_f if d == 0 else out_b) if last \
                            else h_hist[l][d]
                        if d == 0:
                            roww = g00 + half * U
                        else:
                            roww = (S - (1 + half) * U) - g00
                        dyn_dma(nc,
                            dst[:, ds(roww, U), :], hst[d][hp][:])
    nc.compile()
    return nc


# ---------------------------------------------------------------------------
# Host side
# ---------------------------------------------------------------------------

def _patchify(x, p=16):
    b, c, W, Hh = x.shape
    w, h = W // p, Hh // p
    x = x.reshape(b, c, w, p, h, p)
    x = x.transpose(0, 2, 4, 3, 5, 1)
    return x.reshape(b, w * h, p * p * c)


def pack_weights(W_ih, W_hh, b_ih, b_hh, BL):
    wihT = np.ascontiguousarray(
        np.asarray(W_ih, np.float32).transpose(0, 1, 3, 2))
    whhT = np.ascontiguousarray(
        np.asarray(W_hh, np.float32).transpose(0, 1, 3, 2)).astype(
            ml_dtypes.bfloat16)
    b_ih = np.asarray(b_ih, np.float32)
    b_hh = np.asarray(b_hh, np.float32)
    brz = b_ih[:, :, :2 * H] + b_hh[:, :, :2 * H]
    bn_i = b_ih[:, :, 2 * H:]
    bias9 = np.concatenate(
        [brz.reshape(DEPTH, 2, 6, P), bn_i.reshape(DEPTH, 2, 3, P)], axis=2)
    bias9 = np.ascontiguousarray(bias9.transpose(0, 1, 3, 2))  # [l,d,p,m]
    bhn = b_hh[:, :, 2 * H:].reshape(DEPTH, 2, 3, 1, P)
    bhnb = np.broadcast_to(bhn, (DEPTH, 2, 3, BL, P)).reshape(
        DEPTH, 2, 3 * BL, P)
    bhnb = np.ascontiguousarray(bhnb.transpose(0, 1, 3, 2))    # [l,d,p,c*BL+b]
    return wihT, whhT, bias9, bhnb


def host_inputs(x, W_ih, W_hh, b_ih, b_hh, n_cores=N_CORES):
    patches = _patchify(np.asarray(x, np.float32))      # (B, S, D)
    B = patches.shape[0]
    BL = B // n_cores
    wihT, whhT, bias9, bhnb = pack_weights(W_ih, W_hh, b_ih, b_hh, BL)
    in_maps = []
    for cc in range(n_cores):
        pt = patches[cc * BL:(cc + 1) * BL]             # (BL, S, D)
        ptT = np.ascontiguousarray(pt.transpose(2, 1, 0)).reshape(D, -1)
        in_maps.append({
            "patchesT": ptT,
            "wihT": wihT,
            "whhT": whhT,
            "bias9": bias9,
            "bhnb": bhnb,
        })
    return in_maps


def unpack_out(res, S, BL):
    """Device out layout [p, s, c*BL+b] -> (BL, S, H)."""
    o = np.asarray(res).reshape(P, S, HC_N, BL)
    return np.ascontiguousarray(o.transpose(3, 1, 2, 0)).reshape(BL, S, H)


_CACHE = {}


def make_jit_runner(nc, n_cores=N_CORES):
    """Persistent jitted SPMD runner (the stock run_bass_via_pjrt re-traces
    and recompiles on every call).  Returns (runner, stage, execute):
    runner(in_maps) -> list of per-core output dicts."""
    import jax
    import numpy as np
    from jax.sharding import Mesh, PartitionSpec, NamedSharding
    from jax.experimental.shard_map import shard_map
    import concourse.mybir as mybir
    from concourse import bass2jax

    bass2jax.install_neuronx_cc_hook()
    assert nc.partition_id_tensor is None or True

    part_name = nc.partition_id_tensor.name if nc.partition_id_tensor else None
    in_names, out_names, out_avals, zero_outs = [], [], [], []
    for alloc in nc.m.functions[0].allocations:
        if not isinstance(alloc, mybir.MemoryLocationSet):
            continue
        name = alloc.memorylocations[0].name
        if alloc.kind == "ExternalInput":
            if name != part_name:
                in_names.append(name)
        elif alloc.kind == "ExternalOutput":
            out_names.append(name)
            shape = tuple(alloc.tensor_shape)
            dtype = mybir.dt.np(alloc.dtype)
            out_avals.append(jax.core.ShapedArray(shape, dtype))
            zero_outs.append(np.zeros(shape, dtype))
    n_params = len(in_names)
    all_names = in_names + out_names
    if part_name is not None:
        all_names = all_names + [part_name]
    donate = tuple(range(n_params, n_params + len(out_names)))

    def _body(*args):
        operands = list(args)
        if part_name is not None:
            operands.append(bass2jax.partition_id_tensor())
        outs = bass2jax._bass_exec_p.bind(
            *operands,
            out_avals=tuple(out_avals),
            in_names=tuple(all_names),
            out_names=tuple(out_names),
            lowering_input_output_aliases=(),
            sim_require_finite=True,
            sim_require_nnan=True,
            nc=nc,
        )
        return tuple(outs)

    devices = jax.devices()[:n_cores]
    mesh = Mesh(np.asarray(devices), ("core",))
    spec = NamedSharding(mesh, PartitionSpec("core"))
    n_all = n_params + len(out_names)
    fn = jax.jit(
        shard_map(_body, mesh=mesh,
                  in_specs=(PartitionSpec("core"),) * n_all,
                  out_specs=(PartitionSpec("core"),) * len(out_names),
                  check_rep=False),
        donate_argnums=donate, keep_unused=True)

    def stage_inputs(in_maps):
        concat = [np.concatenate([np.asarray(m[name]) for m in in_maps], axis=0)
                  for name in in_names]
        return [jax.device_put(a, spec) for a in concat]

    def stage_zeros():
        return [jax.device_put(
            np.zeros((n_cores * z.shape[0], *z.shape[1:]), z.dtype), spec)
            for z in zero_outs]

    def execute(staged_in, staged_zeros):
        return fn(*staged_in, *staged_zeros)

    def runner(in_maps):
        out_arrs = execute(stage_inputs(in_maps), stage_zeros())
        res = []
        for c in range(n_cores):
            res.append({
                name: np.asarray(out_arrs[i]).reshape(
                    n_cores, *out_avals[i].shape)[c]
                for i, name in enumerate(out_names)})
        return res

    runner.stage_inputs = stage_inputs
    runner.stage_zeros = stage_zeros
    runner.execute = execute
    return runner


def get_runner():
    """Build + compile once; returns a callable over in_maps."""
    if "runner" not in _CACHE:
        nc = build_kernel()
        _CACHE["nc"] = nc
        _CACHE["runner"] = make_jit_runner(nc)
    return _CACHE["runner"]


def kernel(x, W_ih, W_hh, b_ih, b_hh):
    in_maps = host_inputs(x, W_ih, W_hh, b_ih, b_hh)
    results = get_runner()(in_maps)
    BL = B_FULL // N_CORES
    y = np.empty((B_FULL, S_FULL, 2 * H), np.float32)
    for cc in range(N_CORES):
        y[cc * BL:(cc + 1) * BL, :, :H] = unpack_out(
            results[cc]["out_f"], S_FULL, BL)
        y[cc * BL:(cc + 1) * BL, :, H:] = unpack_out(
            results[cc]["out_b"], S_FULL, BL)
    return y
